# revision 18
# baseline (speedup 1.0000x reference)
"""Biaffine label attention kernel for 8 trn2 NeuronCores.

out[b, l, i, j] = (head[b] @ W_head.T)[i, l] + (dep[b] @ W_dep.T)[j, l] + bias[l]

with head/dep: [8, 512, 512] f32, label_W: [64, 1024], label_b: [64],
out: [8, 64, 512, 512] f32 (512 MB).

Sharding: data-parallel over batch; core b computes the contiguous 64 MB
slice out[b].  The kernel is output-write bound (~358 GB/s per core), so the
device program is organized to keep the output DMAs maximally efficient
(1 MB dma_starts with 8 KB contiguous DRAM runs) while TensorE / ScalarE /
VectorE generate tiles well under the DMA rate:

  - TensorE: tiny projections h = head@Wh^T (in a swizzled layout) and
    d = dep@Wd^T + b, then one K=1 ones-matmul per label to broadcast
    d[l, :] across 128 partitions into PSUM.
  - ScalarE: PSUM -> SBUF copy of the broadcast row block.
  - VectorE: 4x tensor_scalar_add per label (fp32 2x mode, SBUF->SBUF):
    out_tile[p, c*512 + j] = d[l, j] + h[l, 4p + c].
  - One 1 MB HWDGE DMA per label writes out[b, l] (partition p holds rows
    4p..4p+3 -> 8 KB contiguous runs in DRAM).
"""

import os
import sys
from contextlib import ExitStack

for _p in ("/opt/trn_rl_repo",):
    if os.path.isdir(_p) and _p not in sys.path:
        sys.path.insert(0, _p)

import numpy as np

import concourse.bass as bass
import concourse.bacc as bacc
import concourse.tile as tile
from concourse import mybir
from concourse.bass_utils import run_bass_kernel_spmd

B = 8
S = 512
D = 512
L = 64
KT = D // 128  # contraction tiles
C = S // 128   # i-rows packed per partition
F32 = mybir.dt.float32

_NC_CACHE = None


def _build_nc():
    nc = bacc.Bacc(
        "TRN2", target_bir_lowering=False, debug=False, num_devices=B
    )
    BF16 = mybir.dt.bfloat16
    headT = nc.declare_dram_parameter("headT", [128, KT * S], F32, isOutput=False)
    depT = nc.declare_dram_parameter("depT", [128, KT * S], F32, isOutput=False)
    whT = nc.declare_dram_parameter("whT", [128, KT * L], F32, isOutput=False)
    wdT = nc.declare_dram_parameter("wdT", [128, KT * L], F32, isOutput=False)
    biasv = nc.declare_dram_parameter("biasv", [L, 1], F32, isOutput=False)
    out = nc.declare_dram_parameter("out", [L, S, S], F32, isOutput=True)

    with tile.TileContext(nc) as tc, ExitStack() as ctx:
        const = ctx.enter_context(tc.tile_pool(name="const", bufs=1))
        psum_bc = ctx.enter_context(tc.tile_pool(name="psum_bc", bufs=6, space="PSUM"))
        psum_hd = ctx.enter_context(tc.tile_pool(name="psum_hd", bufs=1, space="PSUM"))
        out_pool = ctx.enter_context(tc.tile_pool(name="outp", bufs=6))

        # All input loads go through the scalar-engine HWDGE ring so the sync
        # ring (and its sequencer) is dedicated to output writes.
        # d-path inputs first: the whole kernel is gated on d' being ready.
        wd = const.tile([128, KT * L], F32)
        nc.scalar.dma_start(wd[:], wdT[:, :])
        bcol = const.tile([L, 1], F32)
        nc.scalar.dma_start(bcol[:], biasv[:, :])
        # dT in per-kt chunks so the d matmul starts before the full 1 MB lands
        dT = []
        for kt in range(KT):
            t = const.tile([128, S], F32, tag=f"dT{kt}")
            nc.scalar.dma_start(t[:], depT[:, kt * S : (kt + 1) * S])
            dT.append(t)
        wh = const.tile([128, KT * L], F32)
        nc.scalar.dma_start(wh[:], whT[:, :])
        hT = const.tile([128, KT * S], F32)
        nc.scalar.dma_start(hT[:], headT[:, :])

        ones2 = const.tile([2, 128], BF16)
        nc.vector.memset(ones2[:], 1.0)

        # d'[l, j] = sum_d dep[j, d] * W_dep[l, d] + b[l]   (l on partitions)
        dps = psum_hd.tile([L, S], F32)
        for kt in range(KT):
            nc.tensor.matmul(
                dps[:],
                wd[:, kt * L : (kt + 1) * L],
                dT[kt][:],
                start=(kt == 0),
                stop=(kt == KT - 1),
            )
        d_sb = const.tile([L, S], F32)
        nc.scalar.add(d_sb[:], dps[:], bcol[:])

        # Split d' into bf16 hi + lo so the per-label broadcast matmul runs at
        # full PE rate (fp32 matmul is ~8x slower); hi+lo recovers ~fp32
        # precision since PSUM accumulates in fp32.
        d_hi = const.tile([L, S], BF16)
        nc.vector.tensor_copy(d_hi[:], d_sb[:])
        d_hi32 = const.tile([L, S], F32)
        nc.scalar.copy(d_hi32[:], d_hi[:])
        d_lo = const.tile([L, S], BF16)
        nc.vector.tensor_sub(d_lo[:], d_sb[:], d_hi32[:])

        # Flatten [L, S] (l on partitions) -> rows of one [2, L*S] tile so the
        # broadcast rhs [2, S] can be sliced at partition base 0 for any l
        # (engine APs may only start at partition 0/32/64).
        d2 = const.tile([2, L * S], BF16)
        d2v = d2[:].rearrange("p (l j) -> p l j", l=L)
        nc.gpsimd.dma_start(d2v[0:1, :, :], d_hi[:])
        nc.gpsimd.dma_start(d2v[1:2, :, :], d_lo[:])

        # h_sw[p, c*L + l] = sum_d head[4p + c, d] * W_head[l, d]
        # headT arrives host-swizzled so lhsT slices are contiguous.
        hps = psum_hd.tile([128, C * L], F32)
        for c in range(C):
            for kt in range(KT):
                nc.tensor.matmul(
                    hps[:, c * L : (c + 1) * L],
                    hT[:, kt * S + c * 128 : kt * S + (c + 1) * 128],
                    wh[:, kt * L : (kt + 1) * L],
                    start=(kt == 0),
                    stop=(kt == KT - 1),
                )
        h_sw = const.tile([128, C * L], F32)
        nc.scalar.copy(h_sw[:], hps[:])

        # out[l, 4p + c, j] = d'[l, j] + h_sw[p, c*L + l]
        # Two labels per SBUF tile -> one 2 MB output DMA per pair.
        out_r = out[:, :, :].rearrange("(lp m) (p c) j -> lp p m (c j)", m=2, c=C)
        for lp in range(L // 2):
            ot = out_pool.tile([128, 2 * C * S], F32)
            for m in range(2):
                l = 2 * lp + m
                bcp = psum_bc.tile([128, S], F32)
                nc.tensor.matmul(
                    bcp[:], ones2[:], d2v[:, l, :], start=True, stop=True
                )
                for c in range(C):
                    scalar = h_sw[:, c * L + l : c * L + l + 1]
                    dst = ot[:, (m * C + c) * S : (m * C + c + 1) * S]
                    if c < 2:
                        nc.vector.tensor_scalar_add(dst, bcp[:], scalar)
                    else:
                        nc.scalar.add(dst, bcp[:], scalar)
            nc.sync.dma_start(out_r[lp], ot[:])
    nc.compile()
    return nc


def _row_tile(a):
    """[D, F] -> [128, KT*F]: row d = kt*128 + p lands at [p, kt*F : (kt+1)*F]."""
    d, f = a.shape
    kt = d // 128
    return np.ascontiguousarray(
        a.reshape(kt, 128, f).transpose(1, 0, 2).reshape(128, kt * f)
    )


def _prep_inputs(head, dep, label_W, label_b):
    head = np.asarray(head, dtype=np.float32)
    dep = np.asarray(dep, dtype=np.float32)
    label_W = np.asarray(label_W, dtype=np.float32)
    label_b = np.asarray(label_b, dtype=np.float32)

    wh = _row_tile(np.ascontiguousarray(label_W[:, :D].T))  # [128, KT*L]
    wd = _row_tile(np.ascontiguousarray(label_W[:, D:].T))
    bias = np.ascontiguousarray(label_b.reshape(L, 1))

    in_maps = []
    for b in range(B):
        ht = head[b].T  # [D, S]
        # column swizzle: ht_sw[d, c*128 + m] = ht[d, 4m + c]
        ht_sw = ht.reshape(D, S // C, C).transpose(0, 2, 1).reshape(D, S)
        in_maps.append(
            {
                "headT": _row_tile(ht_sw),
                "depT": _row_tile(np.ascontiguousarray(dep[b].T)),
                "whT": wh,
                "wdT": wd,
                "biasv": bias,
            }
        )
    return in_maps


def _run(head, dep, label_W, label_b, trace=False, **trace_kwargs):
    global _NC_CACHE
    if _NC_CACHE is None:
        _NC_CACHE = _build_nc()
    in_maps = _prep_inputs(head, dep, label_W, label_b)
    res = run_bass_kernel_spmd(
        _NC_CACHE, in_maps, list(range(B)), trace=trace, **trace_kwargs
    )
    out = np.stack([res.results[i]["out"] for i in range(B)])
    return out, res


def kernel(head, dep, label_W, label_b):
    out, _ = _run(head, dep, label_W, label_b, trace=False)
    return out
